# revision 14
# baseline (speedup 1.0000x reference)
# Malvar demosaic on 8 Trainium2 NeuronCores — pure data parallel (1 batch
# image per core).
#
# Strategy: polyphase decomposition. Each output (channel, Bayer-parity)
# plane at quarter resolution is a short sum of terms
#   (input phase, horizontal phase-shift) x (vertical 3-tap band),
# computed as banded [128 x 126] fp32r matmuls on the TensorEngine (vertical
# mixing across partitions) with horizontal shifts expressed as strided rhs
# column reads. Reflection padding is folded into the band matrices of the
# first/last row blocks; the 4 passthrough planes route through PE as
# identity bands (PE is the only engine that can shift across partitions).
# DVE clips conv results to [0,1] while interleaving even/odd output columns
# into assembled full-resolution row tiles; ScalarE copies passthroughs.
# All HBM traffic is contiguous >= 8KB per partition line.
import numpy as np
from contextlib import ExitStack


# ---------------------------------------------------------------------------
# Problem constants (hardcoded per harness contract)
B, H, W = 8, 2048, 2048
N_CORES = 8


def MALVAR_KERNELS():
    g = np.array([[0, 0, -1, 0, 0], [0, 0, 2, 0, 0], [-1, 2, 4, 2, -1],
                  [0, 0, 2, 0, 0], [0, 0, -1, 0, 0]], np.float32) / 8.0
    col = np.array([[0, 0, 0.5, 0, 0], [0, -1, 0, -1, 0], [-1, 4, 5, 4, -1],
                    [0, -1, 0, -1, 0], [0, 0, 0.5, 0, 0]], np.float32) / 8.0
    row = np.array([[0, 0, -1, 0, 0], [0, -1, 4, -1, 0], [0.5, 0, 5, 0, 0.5],
                    [0, -1, 4, -1, 0], [0, 0, -1, 0, 0]], np.float32) / 8.0
    br = np.array([[0, 0, -1.5, 0, 0], [0, 2, 0, 2, 0], [-1.5, 0, 6, 0, -1.5],
                   [0, 2, 0, 2, 0], [0, 0, -1.5, 0, 0]], np.float32) / 8.0
    return {"g": g, "col": col, "row": row, "br": br}


# (out channel, row parity di0, col parity dj0, kernel name)
CONV_OUTPUTS = [
    (1, 0, 0, "g"),    # green at R
    (2, 0, 0, "br"),   # blue  at R
    (0, 0, 1, "col"),  # red   at Gr
    (2, 0, 1, "row"),  # blue  at Gr
    (0, 1, 0, "row"),  # red   at Gb
    (2, 1, 0, "col"),  # blue  at Gb
    (0, 1, 1, "br"),   # red   at B
    (1, 1, 1, "g"),    # green at B
]
# passthrough planes: out[ch, 2i+di0, 2j+dj0] = x[2i+di0, 2j+dj0]
PASSTHROUGH_OUTPUTS = [(0, 0, 0), (1, 0, 1), (1, 1, 0), (2, 1, 1)]


def gen_passes(kernels=None):
    """Polyphase decomposition of each output plane.

    Returns a list of 12 dicts {ch, di0, dj0, is_pass, passes} where passes
    is a list of {pr, pc, dcol, taps: {drow: coeff}}. Output plane value:
      out[i, j] = sum over passes, taps:
          coeff * phase[pr,pc][i + drow, j + dcol]
    for output full-res site (2i + di0, 2j + dj0).
    """
    if kernels is None:
        kernels = MALVAR_KERNELS()
    qs = []
    for ch, di0, dj0, kname in CONV_OUTPUTS:
        k = kernels[kname]
        groups = {}
        for u in range(-2, 3):
            for v in range(-2, 3):
                c = float(k[u + 2, v + 2])
                if c == 0.0:
                    continue
                pr = (di0 + u) % 2
                drow = (di0 + u - pr) // 2
                pc = (dj0 + v) % 2
                dcol = (dj0 + v - pc) // 2
                key = (pr, pc, dcol)
                groups.setdefault(key, {})
                groups[key][drow] = groups[key].get(drow, 0.0) + c
        passes = [{"pr": pr, "pc": pc, "dcol": dcol, "taps": taps}
                  for (pr, pc, dcol), taps in sorted(groups.items())]
        qs.append({"ch": ch, "di0": di0, "dj0": dj0, "is_pass": False,
                   "passes": passes})
    for ch, di0, dj0 in PASSTHROUGH_OUTPUTS:
        qs.append({"ch": ch, "di0": di0, "dj0": dj0, "is_pass": True,
                   "passes": [{"pr": di0, "pc": dj0, "dcol": 0,
                               "taps": {0: 1.0}}]})
    return qs


def block_plan(n):
    """Row-block plan over n phase rows. Returns [(base, out0, M, cls)].

    Block covers output phase rows [out0, out0+M); its input tiles hold
    phase rows [base, base+128). cls: 0 first (reflect top), 1 interior,
    2 last (reflect bottom).
    """
    assert n >= 128
    plan = []
    out0 = 0
    while out0 < n:
        if out0 == 0:
            base, cls, M = 0, 0, 126
        elif out0 <= n - 127:
            base, cls, M = out0 - 1, 1, 126
        else:
            base, cls, M = n - 128, 2, n - out0
        plan.append((base, out0, M, cls))
        out0 += M
    return plan


def _class_geometry(n, cls):
    plan = block_plan(n)
    if cls == 0:
        return plan[0]
    if cls == 2:
        return plan[-1]
    interior = [b for b in plan if b[3] == 1]
    return interior[0] if interior else None


def gen_bands(n, cls, kernels=None):
    """Band (lhsT) matrices [128, 126] for every (q, pass) for block class
    cls. lhsT[k, m] = coeff so that psum[m, :] += sum_k lhsT[k, m]*tile[k, :]
    computes output phase row out0+m from tile rows (phase rows base+k),
    with reflection rows folded in."""
    qs = gen_passes(kernels)
    geo = _class_geometry(n, cls)
    bands = {}
    for qi, q in enumerate(qs):
        for pi, p in enumerate(q["passes"]):
            B = np.zeros((128, 126), np.float32)
            if geo is not None:
                base, out0, M, _ = geo
                pr = p["pr"]
                for m in range(126):
                    if out0 + m >= n:
                        continue
                    for drow, coeff in p["taps"].items():
                        r = out0 + m + drow
                        if r < 0:
                            r = -r - pr          # reflect top (same parity)
                        elif r >= n:
                            r = 2 * n - 1 - r - pr  # reflect bottom
                        k = r - base
                        assert 0 <= k < 128, (cls, qi, pi, m, drow, k)
                        B[k, m] += coeff
            bands[(qi, pi)] = B
    return bands


def build_bands_np(n, kernels=None):
    """[3, 128, NPT*126] f32 band tensor (partition-major for fast DMA)."""
    qs = gen_passes(kernels)
    npt = sum(len(q["passes"]) for q in qs)
    arr = np.zeros((3, 128, npt * 126), np.float32)
    for cls in range(3):
        bands = gen_bands(n, cls, kernels)
        g = 0
        for qi, q in enumerate(qs):
            for pi in range(len(q["passes"])):
                arr[cls, :, g * 126:(g + 1) * 126] = bands[(qi, pi)]
                g += 1
    return np.ascontiguousarray(arr)


# ---------------------------------------------------------------------------
# Bass module
def build_nc(H_, W_, kernels=None, num_devices=N_CORES):
    import concourse.bacc as bacc
    import concourse.tile as tile
    import concourse.mybir as mybir

    F32 = mybir.dt.float32
    F32R = mybir.dt.float32r

    n, wn = H_ // 2, W_ // 2
    NCH = min(512, wn)           # matmul moving free dim (one PSUM bank fp32)
    assert wn % NCH == 0
    nchunks = wn // NCH
    qs = gen_passes(kernels)
    gpi_of = {}
    g = 0
    for qi, q in enumerate(qs):
        for pi in range(len(q["passes"])):
            gpi_of[(qi, pi)] = g
            g += 1
    NPT = g
    plan = block_plan(n)

    nc = bacc.Bacc("TRN2", target_bir_lowering=False, debug=False,
                   enable_asserts=False, num_devices=num_devices)
    # float32r end-to-end on the matmul input path: the PE consumes fp32r
    # (rounded fp32) at full rate; the verifier requires producers typed f32r.
    x = nc.dram_tensor("x", [H_, W_], F32R, kind="ExternalInput").ap()
    bands_d = nc.dram_tensor("bands", [3, 128, NPT * 126], F32R,
                             kind="ExternalInput").ap()
    y = nc.dram_tensor("y", [3, H_, W_], F32, kind="ExternalOutput").ap()

    with ExitStack() as ctx:
        tc = ctx.enter_context(tile.TileContext(nc))
        in_pool = ctx.enter_context(tc.tile_pool(name="inp", bufs=2))
        band_pool = ctx.enter_context(tc.tile_pool(name="band", bufs=2))
        out_pool = ctx.enter_context(tc.tile_pool(name="outp", bufs=2))
        psum_pool = ctx.enter_context(tc.tile_pool(name="ps", bufs=8,
                                                   space="PSUM"))
        band_tiles = {}

        def get_band_tile(cls):
            if cls not in band_tiles:
                bt = band_pool.tile([128, NPT * 126], F32R, tag="bands")
                nc.sync.dma_start(bt[:, :], bands_d[cls])
                band_tiles[cls] = bt
            return band_tiles[cls]

        for (base, out0, M, cls) in plan:
            bt = get_band_tile(cls)
            tin = {}
            for pr in (0, 1):
                t = in_pool.tile([128, W_ + 4], F32R, tag=f"t{pr}")
                nc.sync.dma_start(t[:, 2:W_ + 2],
                                  x[2 * base + pr: 2 * base + pr + 255: 2, :])
                # reflect-pad columns: tile col c <-> image col c-2
                nc.scalar.copy(t[:, 0:1], t[:, 4:5])
                nc.scalar.copy(t[:, 1:2], t[:, 3:4])
                nc.scalar.copy(t[:, W_ + 2:W_ + 3], t[:, W_:W_ + 1])
                nc.scalar.copy(t[:, W_ + 3:W_ + 4], t[:, W_ - 1:W_])
                tin[pr] = t
            A = {(ch, dy): out_pool.tile([128, W_], F32, tag=f"A{ch}{dy}",
                                         name=f"A{ch}{dy}")
                 for ch in range(3) for dy in (0, 1)}
            for qi, q in enumerate(qs):
                ch, di0, dj0 = q["ch"], q["di0"], q["dj0"]
                npass = len(q["passes"])
                for c in range(nchunks):
                    ps = psum_pool.tile([128, NCH], F32, tag="ps")
                    for pi, p in enumerate(q["passes"]):
                        gp = gpi_of[(qi, pi)]
                        lhsT = bt[:, gp * 126: gp * 126 + 126]
                        c0 = 2 * p["dcol"] + p["pc"] + 2 + 2 * NCH * c
                        rhs = tin[p["pr"]][:, c0: c0 + 2 * NCH - 1: 2]
                        nc.tensor.matmul(ps[0:126, :], lhsT, rhs,
                                         start=(pi == 0), stop=(pi == npass - 1))
                    dest = A[(ch, di0)][0:126,
                                        2 * NCH * c + dj0:
                                        2 * NCH * (c + 1) + dj0 - 1: 2]
                    if q["is_pass"]:
                        nc.scalar.copy(dest, ps[0:126, :])
                    else:
                        nc.vector.tensor_scalar(
                            dest, ps[0:126, :], 1.0, 0.0,
                            mybir.AluOpType.min, mybir.AluOpType.max)
            for (ch, dy), t in A.items():
                nc.sync.dma_start(
                    y[ch, 2 * out0 + dy: 2 * out0 + dy + 2 * M - 1: 2, :],
                    t[0:M, :])
    nc.compile()
    return nc


# ---------------------------------------------------------------------------
_NC_CACHE = {}


_LAST_RESULTS = None


def kernel(**inputs) -> np.ndarray:
    import os
    from concourse import bass_utils

    bayer = np.asarray(inputs["bayer"], dtype=np.float32)
    b, c1, h, w = bayer.shape
    assert (b, c1, h, w) == (B, 1, H, W), bayer.shape

    kernels = None
    if "k_g_at_rb" in inputs:
        kernels = {
            "g": np.asarray(inputs["k_g_at_rb"], np.float32).reshape(5, 5),
            "col": np.asarray(inputs["k_rb_at_g_col"], np.float32).reshape(5, 5),
            "row": np.asarray(inputs["k_rb_at_g_row"], np.float32).reshape(5, 5),
            "br": np.asarray(inputs["k_rb_at_br"], np.float32).reshape(5, 5),
        }

    key = (h, w)
    if key not in _NC_CACHE:
        _NC_CACHE[key] = build_nc(h, w, kernels)
    nc = _NC_CACHE[key]

    bands_np = build_bands_np(h // 2, kernels)
    in_maps = [{"x": np.ascontiguousarray(bayer[i, 0]), "bands": bands_np}
               for i in range(N_CORES)]
    trace = os.environ.get("DEMOSAIC_TRACE", "0") == "1"
    res = bass_utils.run_bass_kernel_spmd(nc, in_maps,
                                          core_ids=list(range(N_CORES)),
                                          trace=trace)
    global _LAST_RESULTS
    _LAST_RESULTS = res
    out = np.stack([r["y"] for r in res.results], axis=0)
    return out.astype(np.float32, copy=False)


if __name__ == "__main__":
    # smoke: band/pass structure
    qs = gen_passes()
    for q in qs:
        print(q["ch"], q["di0"], q["dj0"], "passes:", len(q["passes"]),
              "pass" if q["is_pass"] else "conv")
    print("total passes:", sum(len(q["passes"]) for q in qs))
    print("plan n=1024:", block_plan(1024))
